# revision 1
# baseline (speedup 1.0000x reference)
"""CrossAttentionHook kernel for 8x Trainium2 NeuronCores (Bass/Tile).

Sharding: 8 cores = 4 batches x 2 query-halves; each core independently
computes 1024 query rows end-to-end (no collectives).

Layout strategy: everything flows feature-major (x.T [features, tokens]);
weights are host-pre-transposed so every GEMM operand loads naturally with
the contraction dim on SBUF partitions. The kv projection is folded into
wk/wv on the host (k = aug @ (wk@proj_w).T + (wk@proj_b + bk)), which
removes one full GEMM phase. Attention computes transposed scores
P.T[ktok, qtok] per head (DH=128 = one partition block), exp on ScalarE
(no max-subtraction needed: scores ~ N(0,1)), denominators via an
all-ones [128,128] stationary matmul (output rows pre-replicated across
partitions), V kept token-major straight from its projection. All matmuls
run as float32r (4-byte streaming mode, ~3x bf16 rate at N=512).
"""
import sys

sys.path.insert(0, "/opt/trn_rl_repo")

import numpy as np

import concourse.bass as bass  # noqa: F401  (registers engine types)
import concourse.mybir as mybir
from concourse import bacc
from concourse.tile import TileContext

B, LQ, LKV, E, AUG, H, DH = 4, 2048, 2048, 2048, 1024, 16, 128
EPS = 1e-6
QT = LQ // 2          # per-core query tokens
N_CORES = 8
F32 = mybir.dt.float32
F32R = mybir.dt.float32r
AF = mybir.ActivationFunctionType
ALU = mybir.AluOpType

KC_E = E // 128       # 16 chunks over E
KC_A = AUG // 128     # 8 chunks over AUG
NQ = QT // 512        # 2 query-token chunks of 512
NK = LKV // 512       # 4 key-token chunks of 512
KT = LKV // 128       # 16 key-token chunks of 128


def build(reps: int = 1):
    nc = bacc.Bacc("TRN2", target_bir_lowering=False, debug=False,
                   num_devices=N_CORES)

    qT = nc.dram_tensor("qT", [E, QT], F32R, kind="ExternalInput")
    augT = nc.dram_tensor("augT", [AUG, LKV], F32R, kind="ExternalInput")
    wqT = nc.dram_tensor("wqT", [E, E], F32R, kind="ExternalInput")
    wkeT = nc.dram_tensor("wkeT", [AUG, E], F32R, kind="ExternalInput")
    wveT = nc.dram_tensor("wveT", [AUG, E], F32R, kind="ExternalInput")
    woT = nc.dram_tensor("woT", [E, E], F32R, kind="ExternalInput")
    ones_d = nc.dram_tensor("ones_d", [128, 128], F32R, kind="ExternalInput")
    bvr_d = nc.dram_tensor("bvr", [128, E], F32, kind="ExternalInput")
    bq_d = nc.dram_tensor("bq", [E], F32, kind="ExternalInput")
    bk_d = nc.dram_tensor("bk", [E], F32, kind="ExternalInput")
    bv_d = nc.dram_tensor("bv", [E], F32, kind="ExternalInput")
    bo_d = nc.dram_tensor("bo", [E], F32, kind="ExternalInput")
    gn_d = nc.dram_tensor("gn", [E], F32, kind="ExternalInput")
    outT = nc.dram_tensor("outT", [E, QT], F32, kind="ExternalOutput")

    with TileContext(nc) as tc:
        with (
            tc.tile_pool(name="cst", bufs=1) as cpool,
            tc.tile_pool(name="bias", bufs=16) as bpool,
            tc.tile_pool(name="ctx", bufs=16) as xpool,
            tc.tile_pool(name="psproj", bufs=2, space="PSUM") as pj,
            tc.tile_pool(name="pssc", bufs=2, space="PSUM") as psc,
            tc.tile_pool(name="psdn", bufs=2, space="PSUM") as pdn,
            tc.tile_pool(name="psctx", bufs=2, space="PSUM") as pcx,
            tc.tile_pool(name="dram", bufs=1, space="DRAM") as dpool,
        ):
            kT_d = dpool.tile([E, LKV], F32R, tag="k_spill")
            v_d = dpool.tile([LKV, E], F32R, tag="v_spill")
            qh_d = dpool.tile([E, QT], F32R, tag="q_spill")

            ones = cpool.tile([128, 128], F32R, tag="ones")
            nc.sync.dma_start(out=ones[:], in_=ones_d[:])
            bias = {}
            for nm, dr in (("bq", bq_d), ("bk", bk_d), ("bv", bv_d),
                           ("bo", bo_d), ("gn", gn_d)):
                for m in range(KC_E):
                    t = bpool.tile([128, 1], F32, tag=nm)
                    nc.sync.dma_start(
                        out=t[:], in_=dr[m * 128:(m + 1) * 128].unsqueeze(1))
                    bias[nm, m] = t

            def body(iv=None):
                # ---- Phase A: q-proj -> spill q_hat.T to DRAM ----
                with (
                    tc.tile_pool(name="qin", bufs=16) as qpool,
                    tc.tile_pool(name="wcolA", bufs=3) as wcpool,
                    tc.tile_pool(name="qout", bufs=3) as qopool,
                ):
                    qin = []
                    for k in range(KC_E):
                        t = qpool.tile([128, QT], F32R, tag="qin")
                        nc.sync.dma_start(out=t[:], in_=qT[k * 128:(k + 1) * 128, :])
                        qin.append(t)
                    for m in range(KC_E):
                        wc = wcpool.tile([128, E], F32R, tag="w")
                        nc.sync.dma_start(
                            out=wc[:].rearrange("p (k c) -> p k c", c=128),
                            in_=wqT[:, m * 128:(m + 1) * 128]
                            .rearrange("(k p) c -> p k c", p=128))
                        qh = qopool.tile([128, QT], F32R, tag="qh")
                        for n in range(NQ):
                            ps = pj.tile([128, 512], F32, tag="mm")
                            for k in range(KC_E):
                                nc.tensor.matmul(
                                    ps[:], wc[:, k * 128:(k + 1) * 128],
                                    qin[k][:, n * 512:(n + 1) * 512],
                                    start=(k == 0), stop=(k == KC_E - 1))
                            nc.vector.tensor_scalar_add(
                                qh[:, n * 512:(n + 1) * 512], ps[:],
                                bias["bq", m][:])
                        nc.scalar.dma_start(
                            out=qh_d[m * 128:(m + 1) * 128, :], in_=qh[:])

                # ---- Phases B & C: k-proj / v-proj (kv folded in), spill ----
                with (
                    tc.tile_pool(name="aug", bufs=8) as gpool,
                    tc.tile_pool(name="wcolB", bufs=3) as wcpool,
                    tc.tile_pool(name="kout", bufs=2) as kopool,
                    tc.tile_pool(name="wvcol", bufs=10) as wvpool,
                    tc.tile_pool(name="vout", bufs=3) as vopool,
                ):
                    augin = []
                    for k in range(KC_A):
                        t = gpool.tile([128, LKV], F32R, tag="aug")
                        nc.sync.dma_start(
                            out=t[:], in_=augT[k * 128:(k + 1) * 128, :])
                        augin.append(t)
                    for m in range(KC_E):
                        wc = wcpool.tile([128, AUG], F32R, tag="w")
                        nc.sync.dma_start(
                            out=wc[:].rearrange("p (k c) -> p k c", c=128),
                            in_=wkeT[:, m * 128:(m + 1) * 128]
                            .rearrange("(k p) c -> p k c", p=128))
                        ks = kopool.tile([128, LKV], F32R, tag="ks")
                        for n in range(NK):
                            ps = pj.tile([128, 512], F32, tag="mm")
                            for k in range(KC_A):
                                nc.tensor.matmul(
                                    ps[:], wc[:, k * 128:(k + 1) * 128],
                                    augin[k][:, n * 512:(n + 1) * 512],
                                    start=(k == 0), stop=(k == KC_A - 1))
                            nc.vector.tensor_scalar_add(
                                ks[:, n * 512:(n + 1) * 512], ps[:],
                                bias["bk", m][:])
                        nc.scalar.dma_start(
                            out=kT_d[m * 128:(m + 1) * 128, :], in_=ks[:])
                    # v-proj: token-major output, wve column-blocks streamed
                    for n in range(KC_E // 4):
                        vb = vopool.tile([128, 512], F32, tag="vb")
                        nc.sync.dma_start(
                            out=vb[:], in_=bvr_d[:, n * 512:(n + 1) * 512])
                        wv = []
                        for k in range(KC_A):
                            t = wvpool.tile([128, 512], F32R, tag="wv")
                            nc.sync.dma_start(
                                out=t[:],
                                in_=wveT[k * 128:(k + 1) * 128,
                                         n * 512:(n + 1) * 512])
                            wv.append(t)
                        for kt in range(KT):
                            ps = pj.tile([128, 512], F32, tag="mm")
                            for k in range(KC_A):
                                nc.tensor.matmul(
                                    ps[:], augin[k][:, kt * 128:(kt + 1) * 128],
                                    wv[k][:],
                                    start=(k == 0), stop=(k == KC_A - 1))
                            vo = vopool.tile([128, 512], F32R, tag="vo")
                            nc.vector.tensor_tensor(
                                vo[:], ps[:], vb[:], ALU.add)
                            nc.scalar.dma_start(
                                out=v_d[kt * 128:(kt + 1) * 128,
                                        n * 512:(n + 1) * 512],
                                in_=vo[:])

                # ---- Phase D: attention per head ----
                ctxT = []
                with (
                    tc.tile_pool(name="kv", bufs=2) as hpool,
                    tc.tile_pool(name="qhs", bufs=2) as qspool,
                    tc.tile_pool(name="expp", bufs=3) as epool,
                    tc.tile_pool(name="rec", bufs=2) as rpool,
                ):
                    for h in range(H):
                        kh = hpool.tile([128, LKV], F32R, tag="kh")
                        nc.sync.dma_start(
                            out=kh[:], in_=kT_d[h * 128:(h + 1) * 128, :])
                        vh = hpool.tile([128, LKV], F32R, tag="vh")
                        nc.sync.dma_start(
                            out=vh[:].rearrange("p (t d) -> p t d", d=128),
                            in_=v_d[:, h * 128:(h + 1) * 128]
                            .rearrange("(t p) d -> p t d", p=128))
                        qh = qspool.tile([128, QT], F32R, tag="qh")
                        nc.sync.dma_start(
                            out=qh[:], in_=qh_d[h * 128:(h + 1) * 128, :])
                        ch = xpool.tile([128, QT], F32R, tag="ctx")
                        for n in range(NQ):
                            dn = pdn.tile([128, 512], F32, tag="dn")
                            cx = pcx.tile([128, 512], F32, tag="cx")
                            for kt in range(KT):
                                sc = psc.tile([128, 512], F32, tag="sc")
                                nc.tensor.matmul(
                                    sc[:], kh[:, kt * 128:(kt + 1) * 128],
                                    qh[:, n * 512:(n + 1) * 512],
                                    start=True, stop=True)
                                ep = epool.tile([128, 512], F32R, tag="ep")
                                nc.scalar.activation(ep[:], sc[:], AF.Exp)
                                nc.tensor.matmul(
                                    dn[:], ones[:], ep[:],
                                    start=(kt == 0), stop=(kt == KT - 1))
                                nc.tensor.matmul(
                                    cx[:], vh[:, kt * 128:(kt + 1) * 128], ep[:],
                                    start=(kt == 0), stop=(kt == KT - 1))
                            rec = rpool.tile([128, 512], F32, tag="rec")
                            nc.vector.reciprocal(rec[:], dn[:])
                            nc.vector.tensor_tensor(
                                ch[:, n * 512:(n + 1) * 512], cx[:], rec[:],
                                ALU.mult)
                        ctxT.append(ch)

                # ---- Phase E: out-proj + RMSNorm + residual ----
                with (
                    tc.tile_pool(name="wcolE", bufs=3) as wcpool,
                    tc.tile_pool(name="asb", bufs=17) as apool2,
                    tc.tile_pool(name="fin", bufs=2) as fpool,
                ):
                    for n in range(NQ):
                        asb = []
                        ss = pdn.tile([128, 512], F32, tag="dn")
                        for m in range(KC_E):
                            wc = wcpool.tile([128, E], F32R, tag="w")
                            nc.sync.dma_start(
                                out=wc[:].rearrange("p (k c) -> p k c", c=128),
                                in_=woT[:, m * 128:(m + 1) * 128]
                                .rearrange("(k p) c -> p k c", p=128))
                            ps = pj.tile([128, 512], F32, tag="mm")
                            for k in range(KC_E):
                                nc.tensor.matmul(
                                    ps[:], wc[:, k * 128:(k + 1) * 128],
                                    ctxT[k][:, n * 512:(n + 1) * 512],
                                    start=(k == 0), stop=(k == KC_E - 1))
                            at = apool2.tile([128, 512], F32R, tag="at")
                            nc.vector.tensor_scalar_add(
                                at[:], ps[:], bias["bo", m][:])
                            sq = fpool.tile([128, 512], F32R, tag="sq")
                            nc.scalar.activation(sq[:], at[:], AF.Square)
                            nc.tensor.matmul(ss[:], ones[:], sq[:],
                                             start=(m == 0), stop=(m == KC_E - 1))
                            asb.append(at)
                        t1 = fpool.tile([128, 512], F32, tag="t1")
                        nc.vector.tensor_scalar(t1[:], ss[:], 1.0 / E, EPS,
                                                ALU.mult, ALU.add)
                        t2 = fpool.tile([128, 512], F32, tag="t2")
                        nc.vector.reciprocal(t2[:], t1[:])
                        rstd = fpool.tile([128, 512], F32, tag="rstd")
                        nc.scalar.activation(rstd[:], t2[:], AF.Sqrt)
                        for m in range(KC_E):
                            qe = fpool.tile([128, 512], F32R, tag="qe")
                            nc.sync.dma_start(
                                out=qe[:],
                                in_=qT[m * 128:(m + 1) * 128,
                                       n * 512:(n + 1) * 512])
                            tm = fpool.tile([128, 512], F32, tag="tm")
                            nc.vector.tensor_tensor(
                                tm[:], asb[m][:], rstd[:], ALU.mult)
                            ob = fpool.tile([128, 512], F32, tag="ob")
                            nc.vector.scalar_tensor_tensor(
                                ob[:], tm[:], bias["gn", m][:],
                                qe[:].bitcast(F32), ALU.mult, ALU.add)
                            nc.sync.dma_start(
                                out=outT[m * 128:(m + 1) * 128,
                                         n * 512:(n + 1) * 512],
                                in_=ob[:])

            if reps == 1:
                body()
            else:
                with tc.For_i(0, reps, 1) as iv:
                    body(iv)

    nc.compile()
    return nc


def prep_inputs(query, aug_hidden_state, aug_mask, proj_w, proj_b,
                in_proj_w, in_proj_b, out_proj_w, out_proj_b, rms_w):
    del aug_mask
    f = np.float32
    query = np.asarray(query, f)
    aug = np.asarray(aug_hidden_state, f)
    proj_w = np.asarray(proj_w, f)
    proj_b = np.asarray(proj_b, f)
    in_proj_w = np.asarray(in_proj_w, f)
    in_proj_b = np.asarray(in_proj_b, f)
    out_proj_w = np.asarray(out_proj_w, f)
    out_proj_b = np.asarray(out_proj_b, f)
    rms_w = np.asarray(rms_w, f)

    s = f(1.0 / np.sqrt(DH))
    wq, wk, wv = in_proj_w[:E], in_proj_w[E:2 * E], in_proj_w[2 * E:]
    bq, bk, bv = in_proj_b[:E], in_proj_b[E:2 * E], in_proj_b[2 * E:]
    shared = {
        "wqT": np.ascontiguousarray((wq * s).T),
        "wkeT": np.ascontiguousarray((wk @ proj_w).T),
        "wveT": np.ascontiguousarray((wv @ proj_w).T),
        "woT": np.ascontiguousarray(out_proj_w.T),
        "ones_d": np.ones((128, 128), f),
        "bq": np.ascontiguousarray(bq * s),
        "bk": np.ascontiguousarray(wk @ proj_b + bk),
        "bv": np.ascontiguousarray(wv @ proj_b + bv),
        "bo": np.ascontiguousarray(out_proj_b),
        "bvr": np.ascontiguousarray(
            np.broadcast_to(wv @ proj_b + bv, (128, E)).astype(f)),
        "gn": np.ascontiguousarray(1.0 + rms_w),
    }
    in_maps = []
    for c in range(N_CORES):
        b, half = c // 2, c % 2
        m = dict(shared)
        m["qT"] = np.ascontiguousarray(query[b, half * QT:(half + 1) * QT, :].T)
        m["augT"] = np.ascontiguousarray(aug[b].T)
        in_maps.append(m)
    return in_maps


def assemble(results, dtype):
    out = np.empty((B, LQ, E), np.float32)
    for c in range(N_CORES):
        b, half = c // 2, c % 2
        out[b, half * QT:(half + 1) * QT, :] = results[c]["outT"].T
    return out.astype(dtype, copy=False)


_CACHE = {}


def _get_runner():
    if "runner" not in _CACHE:
        from concourse.bass_utils import run_bass_kernel_spmd  # noqa: F401
        nc = build(reps=1)
        _CACHE["nc"] = nc
        _CACHE["runner"] = True
    return _CACHE["nc"]


def kernel(**inputs):
    nc = _get_runner()
    from concourse.bass_utils import run_bass_kernel_spmd
    in_maps = prep_inputs(**inputs)
    res = run_bass_kernel_spmd(nc, in_maps, list(range(N_CORES)))
    return assemble(res.results, np.asarray(inputs["query"]).dtype)

